# revision 19
# baseline (speedup 1.0000x reference)
"""Trainium2 Bass kernel for BertSelfAttentionSubstitute (relu^2 attention).

Full (unsharded) inputs in, full output out. Internally shards across 8
NeuronCores: data-parallel over batch (B=4) x tensor-parallel over heads
(16 heads -> 2 groups of 8). Core i handles batch b=i//2, heads
8*(i%2)..8*(i%2)+7.

Per-core device program (all shapes hardcoded, fp16 on-chip data):
  inputs:  xt  [1024, 2048]  = hidden[b].T                       (fp16)
           wqt [1024, 512]   = (Wq[rows]/8 ).T  (scale folded)   (fp16)
           wkt [1024, 512]   = Wk[rows].T                        (fp16)
           wvt [1024, 512]   = Wv[rows].T                        (fp16)
  output:  out [512, 2048]   row h*64+d = ctx^T[d, q] for local head h

  Stage B: QT = wqt.T @ xt, KT = wkt.T @ xt  (fp16, d_out major)
           V  = xt.T @ wvt                   ([2048,512] fp16, token major)
  Stage C: heads in PAIRS (A = even local head on partitions 0-63 of the
           qt tile, B = odd head on 64-127), q in halves of 1024.
           EVERY matmul is full 128x128 mode (no PE mode switches):
           - scores head h: lhsT = zero-padded per-head K tile (other
             head's partition rows are 0), rhs = packed qt pair tile; the
             zero rows annihilate the other head's q stream.
           - ctx: lhsT = v pair slice [128,128]; out rows 0-63 are valid
             for head A (rhs=probs_A), rows 64-127 for head B
             (rhs=probs_B); each head gets its own PSUM bank and the
             garbage half is never read. Accumulated over j, emitted one
             iteration behind scores so the PE never waits on the
             elementwise chain.
           relu^2: relu on ACT or DVE (PSUM fp32 -> SBUF fp16), square on
           DVE (2x fp16) or GPSIMD, greedy-balanced.
"""

import sys
import numpy as np

sys.path.insert(0, "/opt/trn_rl_repo")

N_CORES = 8
B, S, D_MODEL = 4, 2048, 1024
NH_LOCAL, HD, DOUT = 8, 64, 512  # per-core heads, head dim, d_out slice
P = 128
DIN_CHUNKS = D_MODEL // P  # 8
DOUT_TILES = DOUT // P  # 4 (= head pairs)
TOKC = 512  # token chunk for projections
NTOKC = S // TOKC  # 4
NK = S // P  # 16 k-tiles
QH = 1024  # q-half width in stage C

# EW cost constants (ns) for the greedy engine balancer; relu/sq are per
# [128,1024] tile, copy per [*,512] fp32 PSUM->SBUF copy.
EW_COST = {
    ("relu", "act"): 1040.0, ("relu", "dve"): 1030.0,
    ("sq", "dve"): 545.0, ("sq", "gp"): 1830.0,
    ("copy", "act"): 610.0, ("copy", "dve"): 660.0,
}
GP_ENABLE = True
STAGES = "BC"  # timing-isolation knob: "B" / "C" / "BC"
QK_COPY_ACT = True   # Q copies on ACT, K on DVE
V_COPY_ACT = False   # V copies on DVE

_CACHE = {}


class _Balancer:
    """Greedy engine load balancer for elementwise work."""

    def __init__(self):
        self.load = {"act": 0.0, "dve": 0.0, "gp": 0.0}

    def pick(self, kind, engines):
        eng = min(engines, key=lambda e: self.load[e] + EW_COST[(kind, e)])
        self.load[eng] += EW_COST[(kind, eng)]
        return eng


def _emit(nc, tc, mybir, xt, wqt, wkt, wvt, out, loop_n=None, seed=None):
    f32 = mybir.dt.float32
    f16 = mybir.dt.float16
    Relu = mybir.ActivationFunctionType.Relu

    with tc.tile_pool(name="persist", bufs=1) as persist, \
         tc.tile_pool(name="xtp", bufs=2) as xtp, \
         tc.tile_pool(name="elem", bufs=4) as elem:

        if seed is not None:
            # timing mode: fill internal DRAM inputs from the small seed
            sx = persist.tile([P, TOKC], f32, tag="seedx", name="seedx")
            sw = persist.tile([P, TOKC], f32, tag="seedw", name="seedw")
            nc.sync.dma_start(sx[:], seed[:, 0:TOKC])
            nc.sync.dma_start(sw[:], seed[:, TOKC:2 * TOKC])
            sxh = persist.tile([P, TOKC], f16, tag="seedxh", name="seedxh")
            nc.vector.tensor_copy(sxh[:], sx[:])
            swh = persist.tile([P, TOKC], f16, tag="seedwh", name="seedwh")
            nc.vector.tensor_copy(swh[:], sw[:])
            for d in range(DIN_CHUNKS):
                for c in range(NTOKC):
                    nc.sync.dma_start(
                        xt[d * P:(d + 1) * P, c * TOKC:(c + 1) * TOKC],
                        sxh[:])
                for wap in (wqt, wkt, wvt):
                    nc.sync.dma_start(wap[d * P:(d + 1) * P, :], swh[:])

        qt_sb = [persist.tile([P, S], f16, tag=f"qt{t}", name=f"qt{t}")
                 for t in range(DOUT_TILES)]
        # per-head zero-padded K tiles: head 2t in rows 0-63 (rows 64-127
        # zero), head 2t+1 in rows 64-127 (rows 0-63 zero)
        ktz = [persist.tile([P, S], f16, tag=f"ktz{h}", name=f"ktz{h}")
               for h in range(NH_LOCAL)]
        v_sb = [persist.tile([P, DOUT], f16, tag=f"v{t}", name=f"v{t}")
                for t in range(S // P)]
        for h in range(NH_LOCAL):
            # zero the dead half once (persists across loop iterations)
            dead = slice(HD, P) if h % 2 == 0 else slice(0, HD)
            nc.vector.memset(ktz[h][dead, :], 0.0)
        if "B" not in STAGES:
            # timing-isolation: init stage-B outputs once, outside the loop
            for tl in qt_sb + v_sb:
                nc.vector.memset(tl[:], 0.01)
            for h in range(NH_LOCAL):
                live = slice(0, HD) if h % 2 == 0 else slice(HD, P)
                nc.vector.memset(ktz[h][live, :], 0.01)

        def body():
            # --- load weights ---
            w_tiles = {}
            for wname, wap in (("q", wqt), ("k", wkt), ("v", wvt)):
                for d in range(DIN_CHUNKS):
                    t = persist.tile([P, DOUT], f16, tag=f"w{wname}{d}",
                                     name=f"w{wname}{d}")
                    nc.scalar.dma_start(t[:], wap[d * P:(d + 1) * P, :])
                    w_tiles[(wname, d)] = t

            # --- Stage B: projections ---
            with tc.tile_pool(name="psA", bufs=2, space="PSUM") as psA:
                for c in range(NTOKC if "B" in STAGES else 0):
                    xtc = []
                    for d in range(DIN_CHUNKS):
                        t = xtp.tile([P, TOKC], f16, tag=f"xt{d}",
                                     name=f"xt{d}")
                        nc.sync.dma_start(
                            t[:], xt[d * P:(d + 1) * P, c * TOKC:(c + 1) * TOKC])
                        xtc.append(t)
                    for wname in ("q", "k"):
                        for tt in range(DOUT_TILES):
                            ps = psA.tile([P, TOKC], f32, tag="proj", name="ps")
                            for d in range(DIN_CHUNKS):
                                nc.tensor.matmul(
                                    ps[:],
                                    lhsT=w_tiles[(wname, d)][:, tt * P:(tt + 1) * P],
                                    rhs=xtc[d][:],
                                    start=(d == 0), stop=(d == DIN_CHUNKS - 1))
                            cs = slice(c * TOKC, (c + 1) * TOKC)
                            if wname == "q":
                                if QK_COPY_ACT:
                                    nc.scalar.copy(qt_sb[tt][:, cs], ps[:])
                                else:
                                    nc.vector.tensor_copy(qt_sb[tt][:, cs], ps[:])
                            else:
                                # split K into per-head zero-padded tiles
                                nc.scalar.copy(
                                    ktz[2 * tt][0:HD, cs], ps[0:HD, :])
                                nc.vector.tensor_copy(
                                    ktz[2 * tt + 1][HD:P, cs], ps[HD:P, :])
                    for tt in range(TOKC // P):
                        ps = psA.tile([P, DOUT], f32, tag="projv", name="psv")
                        for d in range(DIN_CHUNKS):
                            nc.tensor.matmul(
                                ps[:],
                                lhsT=xtc[d][:, tt * P:(tt + 1) * P],
                                rhs=w_tiles[("v", d)][:],
                                start=(d == 0), stop=(d == DIN_CHUNKS - 1))
                        if V_COPY_ACT:
                            nc.scalar.copy(v_sb[c * (TOKC // P) + tt][:], ps[:])
                        else:
                            nc.vector.tensor_copy(
                                v_sb[c * (TOKC // P) + tt][:], ps[:])

            # --- Stage C: attention, head pairs, quad-tiled scores ---
            with tc.tile_pool(name="psS", bufs=1, space="PSUM") as psS, \
                 tc.tile_pool(name="psC", bufs=1, space="PSUM") as psC:
                bal = _Balancer()
                for t in range(DOUT_TILES if "C" in STAGES else 0):
                    qt_p = qt_sb[t]
                    vp = slice(t * P, (t + 1) * P)
                    for qh in range(S // QH):
                        q0 = qh * QH
                        ctx = {}
                        for hk in ("A", "B"):
                            for cc in range(QH // TOKC):
                                ctx[(hk, cc)] = psC.tile(
                                    [P, TOKC], f32, tag=f"c{hk}{cc}",
                                    name=f"c{hk}{cc}")

                        def emit_ctx(j, probs):
                            # full-mode: rows 0-63 valid for A, 64-127 for B
                            for hk in ("A", "B"):
                                for cc in range(QH // TOKC):
                                    nc.tensor.matmul(
                                        ctx[(hk, cc)][:],
                                        lhsT=v_sb[j][:, vp],
                                        rhs=probs[hk][:, cc * TOKC:(cc + 1) * TOKC],
                                        start=(j == 0), stop=(j == NK - 1))

                        pend = None
                        for j in range(NK):
                            k0 = j * P
                            psa = psS.tile([P, QH], f32, tag="sA", name="sA")
                            psb = psS.tile([P, QH], f32, tag="sB", name="sB")
                            for cc in range(QH // TOKC):
                                qs = slice(q0 + cc * TOKC, q0 + (cc + 1) * TOKC)
                                os_ = slice(cc * TOKC, (cc + 1) * TOKC)
                                nc.tensor.matmul(
                                    psa[:, os_], lhsT=ktz[2 * t][:, k0:k0 + P],
                                    rhs=qt_p[:, qs], start=True, stop=True)
                                nc.tensor.matmul(
                                    psb[:, os_],
                                    lhsT=ktz[2 * t + 1][:, k0:k0 + P],
                                    rhs=qt_p[:, qs], start=True, stop=True)
                            if pend is not None:
                                emit_ctx(*pend)
                            probs = {}
                            for hk, ps_t in (("A", psa), ("B", psb)):
                                pr = elem.tile([P, QH], f16, tag="p" + hk,
                                               name="p" + hk)
                                rl = elem.tile([P, QH], f16, tag="r" + hk,
                                               name="r" + hk)
                                if bal.pick("relu", ("act", "dve")) == "act":
                                    nc.scalar.activation(rl[:], ps_t[:], Relu)
                                else:
                                    nc.vector.tensor_scalar_max(
                                        rl[:], ps_t[:], 0.0)
                                sq_engs = ("dve", "gp") if GP_ENABLE else ("dve",)
                                if bal.pick("sq", sq_engs) == "gp":
                                    nc.gpsimd.tensor_mul(pr[:], rl[:], rl[:])
                                else:
                                    nc.vector.tensor_mul(pr[:], rl[:], rl[:])
                                probs[hk] = pr
                            pend = (j, probs)
                        emit_ctx(*pend)
                        ostage = elem.tile([P, QH], f32, tag="ostage", bufs=2,
                                           name="ostage")
                        for hk, vr in (("A", slice(0, HD)), ("B", slice(HD, P))):
                            for cc in range(QH // TOKC):
                                dst = ostage[vr, cc * TOKC:(cc + 1) * TOKC]
                                if bal.pick("copy", ("act", "dve")) == "act":
                                    nc.scalar.copy(dst, ctx[(hk, cc)][vr, :])
                                else:
                                    nc.vector.tensor_copy(
                                        dst, ctx[(hk, cc)][vr, :])
                        nc.scalar.dma_start(
                            out[t * P:(t + 1) * P, q0:q0 + QH], ostage[:])

        if loop_n is not None:
            with tc.For_i(0, loop_n, 1):
                body()
        else:
            body()


def _build(loop_n=None, internal_io=False):
    key = ("nc", loop_n, internal_io)
    if key in _CACHE:
        return _CACHE[key]
    import concourse.tile as tile
    from concourse import bacc, mybir

    f32 = mybir.dt.float32
    f16 = mybir.dt.float16

    nc = bacc.Bacc("TRN2", target_bir_lowering=False, debug=False,
                   num_devices=N_CORES)
    ikind = "Internal" if internal_io else "ExternalInput"
    okind = "ExternalOutput"
    xt = nc.dram_tensor("xt", [D_MODEL, S], f16, kind=ikind).ap()
    wqt = nc.dram_tensor("wqt", [D_MODEL, DOUT], f16, kind=ikind).ap()
    wkt = nc.dram_tensor("wkt", [D_MODEL, DOUT], f16, kind=ikind).ap()
    wvt = nc.dram_tensor("wvt", [D_MODEL, DOUT], f16, kind=ikind).ap()
    out = nc.dram_tensor("out", [DOUT, S], f32, kind=okind).ap()
    seed = None
    if internal_io:
        seed = nc.dram_tensor("seed", [P, 2 * TOKC], f32,
                              kind="ExternalInput").ap()

    with tile.TileContext(nc) as tc:
        _emit(nc, tc, mybir, xt, wqt, wkt, wvt, out, loop_n=loop_n, seed=seed)

    nc.compile()
    _CACHE[key] = nc
    return nc


def _in_maps(hidden_states, Wq, Wk, Wv):
    maps = []
    for i in range(N_CORES):
        b = i // 2
        rows = slice(DOUT * (i % 2), DOUT * (i % 2) + DOUT)
        xt = np.ascontiguousarray(hidden_states[b].T).astype(np.float16)
        maps.append({
            "xt": xt,
            "wqt": (np.ascontiguousarray(Wq[rows].T) / 8.0).astype(np.float16),
            "wkt": np.ascontiguousarray(Wk[rows].T).astype(np.float16),
            "wvt": np.ascontiguousarray(Wv[rows].T).astype(np.float16),
        })
    return maps


def kernel(hidden_states, attention_mask, Wq, bq, Wk, bk, Wv, bv):
    # attention_mask / biases are structurally zero for this problem spec.
    from concourse.bass_utils import run_bass_kernel_spmd

    nc = _build()
    hidden_states = np.asarray(hidden_states, dtype=np.float32)
    maps = _in_maps(hidden_states,
                    np.asarray(Wq, np.float32),
                    np.asarray(Wk, np.float32),
                    np.asarray(Wv, np.float32))
    res = run_bass_kernel_spmd(nc, maps, core_ids=list(range(N_CORES)))
    out = np.empty((B, S, D_MODEL), np.float32)
    for i in range(N_CORES):
        b = i // 2
        cols = slice(DOUT * (i % 2), DOUT * (i % 2) + DOUT)
        out[b, :, cols] = res.results[i]["out"].T
    return out
